# revision 2
# baseline (speedup 1.0000x reference)
"""TRN2 Bass kernel v2 for nn_Attention_21758304322201 (sparse_attention).

Reference computation (B=32, L=2048, D=32, C=20):
    v = vals @ W_v.T
    k = LN(keys @ W_k.T);  q = LN(ques @ W_q.T)
    a = q @ k.T / sqrt(C);  a[masked keys] = -inf
    p = softmax(a);  o = p @ v
    out = LN(o + ques)

v2 restructure (vs the v1 baseline) — targets the dependency stalls that
dominated v1's trace (engines <35% busy on real work):
  * Batches are processed in PAIRS with block-diagonal stationaries: the
    score matmuls contract 64 rows (two 22-dim batch blocks) and cover half
    a key chunk of two batches at once; the o matmuls contract a mixed
    128-key partition layout with a block-diagonal vals/ones stationary.
    Safety: concurrent row-disjoint matmuls writing the SAME PSUM bank fault
    the HW (verified empirically) — here pair A (PE rows 0:64) targets bank
    0 and pair B (rows 64:128) bank 1; same-row matmuls serialize in HW.
  * One exp instruction per j over a [128, 1024] 2-bank scores tile
    (all 4 batches), rotated between ACT (native exp) and DVE (int16
    Schraudolph bf16-bit trick). Pool/GpSimd cannot read PSUM.
  * All row-stat broadcasts (LN rstd of q/k, output-LN mu/var) are produced
    pre-broadcast by 128-column indicator stationaries in the stats matmuls
    (PE stream cost depends only on N) -> no DRAM bounce, no broadcast DMAs.
  * The softmax normalizer rows are replicated 32x for free by ones-column
    blocks in the vals stationary; the s*q residual term is computed on the
    idle Pool engine from a pre-arranged bf16 ques layout (quesT2).
  * Output LN is per-q-tile with centered moments (d1 = z - mu_bc,
    var = E[d1^2]); Ln/Exp are batched across groups of 4 q tiles to avoid
    ACT table-set thrash; g_o folds into the Exp bias (ln g).
  * ques/keys/vals ship as bf16 bits from the host (no on-device f32->bf16
    conversions; projection/stat matmuls stream bf16 anyway).
"""
import math

import numpy as np

from concourse import bacc, bass, bass_utils, tile
from concourse import mybir

dt = mybir.dt
F32 = dt.float32
BF16 = dt.bfloat16
I16 = dt.int16
U16 = dt.uint16
AO = mybir.AluOpType
AF = mybir.ActivationFunctionType

# problem constants (hardcoded per harness contract)
B, LQ, LK, D, C = 32, 2048, 2048, 32, 20
EPS = 1e-5
NCORES = 8
BPC = B // NCORES          # batches per core = 4
CAUG = C + 1               # 21-dim augmented projection (mean fold)
CDIM = C + 2               # +1 guard dim for pad masking
NT = 256                   # q-tile width
NQT = LQ // NT             # 8 q tiles
S20 = math.sqrt(C)
GS = 1.0 / S20             # score scale

# int16 Schraudolph (bf16 bit pattern): bits = round(x * A16 + B16)
A16 = 128.0 / math.log(2.0)
B16 = 127.0 * 128.0 - 5.6          # max rel err ~3.3%, mean ~1.8%

# exp engine rotation per j (one [128,1024] exp per j): 'A' = ACT, 'D' = DVE.
# STRICT alternation across j (including qt boundaries): adjacent j's must
# exp on different engines or the 2-deep scores pipeline serializes.
EXP_PATTERN = ("ADADADADA", "DADADADAD")

PHASES = 3
_cache: dict = {}


def _exp_engine(qt, j):
    pat = EXP_PATTERN[qt % len(EXP_PATTERN)]
    return pat[j % len(pat)]




def build_module(KC: int, reps: int = 1):
    """Build the SPMD bass module for per-core work. KC = padded key count."""
    NJ = KC // 128
    nc = bacc.Bacc("TRN2", target_bir_lowering=False, debug=False,
                   num_devices=NCORES)

    def din(name, shape, dty=F32):
        return nc.dram_tensor(name, shape, dty, kind="ExternalInput").ap()

    quesT_d = din("quesT", [128, LQ], U16)
    quesT2_d = din("quesT2", [128, 2 * LQ], U16)
    keysT_d = din("keysT", [128, KC], U16)
    valsP2_d = din("valsP2", [128, NJ * 512], U16)
    wq_d = din("wq_st", [128, 32])
    wk_d = din("wk_st", [128, 32])
    wvz_d = din("wvz", [128, 256])
    indmu_q_d = din("indmu_q", [128, 128])
    indmu_k_d = din("indmu_k", [128, 128])
    indsq_d = din("indsq", [128, 128])
    indz_mu_d = din("indz_mu", [128, 128])
    indz_sq_d = din("indz_sq", [128, 128])
    glb_d = din("glb", [128, 4])     # cols: g_o, b_o, ln(g_o), (pad)
    qrow1_d = nc.dram_tensor("qrow1", [4, LQ], U16, kind="ExternalInput").ap()
    krow1_d = nc.dram_tensor("krow1", [4, KC], U16, kind="ExternalInput").ap()
    out_d = nc.dram_tensor("out", [128, LQ], F32, kind="ExternalOutput").ap()

    with tile.TileContext(nc) as tc:
        with tc.tile_pool(name="inp", bufs=1) as inp, \
             tc.tile_pool(name="cst", bufs=1) as cst:
            # ---- load inputs (already bf16 where applicable) ----
            quesT_bf = inp.tile([128, LQ], BF16)
            nc.sync.dma_start(quesT_bf[:].bitcast(U16), quesT_d)
            quesT2 = inp.tile([128, 2 * LQ], BF16)
            nc.sync.dma_start(quesT2[:].bitcast(U16), quesT2_d)
            keysT_bf = inp.tile([128, KC], BF16)
            nc.sync.dma_start(keysT_bf[:].bitcast(U16), keysT_d)
            valsP2_bf = inp.tile([128, NJ * 512], BF16)
            nc.sync.dma_start(valsP2_bf[:].bitcast(U16), valsP2_d)
            wq_f = cst.tile([128, 32], F32)
            nc.sync.dma_start(wq_f[:], wq_d)
            wk_f = cst.tile([128, 32], F32)
            nc.sync.dma_start(wk_f[:], wk_d)
            wvz_f = cst.tile([128, 256], F32)
            nc.sync.dma_start(wvz_f[:], wvz_d)
            indmu_q_f = cst.tile([128, 128], F32)
            nc.sync.dma_start(indmu_q_f[:], indmu_q_d)
            indmu_k_f = cst.tile([128, 128], F32)
            nc.sync.dma_start(indmu_k_f[:], indmu_k_d)
            indsq_f = cst.tile([128, 128], F32)
            nc.sync.dma_start(indsq_f[:], indsq_d)
            indz_mu = cst.tile([128, 128], F32)
            nc.sync.dma_start(indz_mu[:], indz_mu_d)
            indz_sq_f = cst.tile([128, 128], F32)
            nc.sync.dma_start(indz_sq_f[:], indz_sq_d)
            glb = cst.tile([128, 4], F32)
            nc.sync.dma_start(glb[:], glb_d)

            # ---- one-time constant prep ----
            wq_bf = cst.tile([128, 32], BF16)
            nc.vector.tensor_copy(wq_bf[:], wq_f[:])
            wk_bf = cst.tile([128, 32], BF16)
            nc.vector.tensor_copy(wk_bf[:], wk_f[:])
            wvz_bf = cst.tile([128, 256], BF16)
            nc.vector.tensor_copy(wvz_bf[:], wvz_f[:])
            indmu_q_bf = cst.tile([128, 128], BF16)
            nc.vector.tensor_copy(indmu_q_bf[:], indmu_q_f[:])
            indmu_k_bf = cst.tile([128, 128], BF16)
            nc.vector.tensor_copy(indmu_k_bf[:], indmu_k_f[:])
            indsq_bf = cst.tile([128, 128], BF16)
            nc.vector.tensor_copy(indsq_bf[:], indsq_f[:])
            indz_sq_bf = cst.tile([128, 128], BF16)
            nc.vector.tensor_copy(indz_sq_bf[:], indz_sq_f[:])
            eps_t = cst.tile([128, 1], F32)
            nc.gpsimd.memset(eps_t[:], EPS)
            # paired-layout k stationary; zero blocks + guard rows persist
            # across reps (per-rep folds only write rows rb+0:21)
            ksc2 = cst.tile([128, 2 * KC], BF16)
            nc.gpsimd.memset(ksc2[:], 0.0)
            for b in range(4):
                rb = 32 * (b % 2) + 64 * (b // 2)
                m = b % 2
                gdst = ksc2[rb + 21:rb + 22, :].rearrange(
                    "p (c k) -> p c k", k=128)[:, :, 64 * m:64 * m + 64]
                nc.sync.dma_start(
                    gdst.bitcast(U16),
                    krow1_d[b:b + 1, :].rearrange("p (c k) -> p c k", k=64))

            pk = dict(
                NJ=NJ, quesT_bf=quesT_bf, quesT2=quesT2, keysT_bf=keysT_bf,
                valsP2_bf=valsP2_bf, ksc2=ksc2,
                wq_bf=wq_bf, wk_bf=wk_bf, wvz_bf=wvz_bf,
                indmu_q_bf=indmu_q_bf, indmu_k_bf=indmu_k_bf,
                indsq_bf=indsq_bf, indz_mu=indz_mu, indz_sq_bf=indz_sq_bf,
                glb=glb, eps_t=eps_t, out_d=out_d,
                qrow1_d=qrow1_d, krow1_d=krow1_d,
            )

            if reps == 1:
                _body(nc, tc, pk)
            elif reps == 0:
                pass
            else:
                with tc.For_i(0, reps, 1):
                    _body(nc, tc, pk)

    nc.compile()
    return nc


class _Ph1Side:
    """One side (q or k) of phase 1, chunk-steppable for k/q interleaving."""

    def __init__(self, nc, pk, wk, sb1, pools, src_bf, w_bf, indmu_bf, L, tg):
        self.nc, self.pk, self.L, self.tg = nc, pk, L, tg
        self.src_bf, self.w_bf, self.indmu_bf = src_bf, w_bf, indmu_bf
        self.hat_bf = wk.tile([128, L], BF16, tag=f"hat{tg}")
        self.sq_bf = sb1.tile([128, L], BF16, tag=f"sq{tg}")
        self.var_sb = sb1.tile([128, L], F32, tag=f"var{tg}")
        self.sb1 = sb1
        self.wk = wk
        self.pools = pools       # shared (projp, mup, sqp, musqp)
        self.chunks = list(range(0, L, 512))

    def step(self, t0):
        nc = self.nc
        w = min(512, self.L - t0)
        projp, mup, sqp, musqp = self.pools
        proj_ps = projp.tile([128, 512], F32, tag="pp")
        for b in range(4):
            nc.tensor.matmul(
                proj_ps[32 * b:32 * b + 32, :w],
                self.w_bf[32 * b:32 * b + 32, :],
                self.src_bf[32 * b:32 * b + 32, t0:t0 + w],
                start=True, stop=True,
                tile_position=(32 * b, 32 * b),
            )
        # PSUM -> SBUF bf16 (ACT) + elementwise square (DVE)
        nc.scalar.copy(self.hat_bf[:, t0:t0 + w], proj_ps[:, :w])
        nc.vector.tensor_tensor(self.sq_bf[:, t0:t0 + w],
                                self.hat_bf[:, t0:t0 + w],
                                self.hat_bf[:, t0:t0 + w], AO.mult)
        mu_ps = mup.tile([128, 512], F32, tag="mu")
        nc.tensor.matmul(mu_ps[:, :w], self.indmu_bf[:],
                         self.hat_bf[:, t0:t0 + w], start=True, stop=True)
        sq_ps = sqp.tile([128, 512], F32, tag="sq")
        nc.tensor.matmul(sq_ps[:, :w], self.pk["indsq_bf"][:],
                         self.sq_bf[:, t0:t0 + w], start=True, stop=True)
        musq = musqp.tile([128, 512], F32, tag="musq")
        nc.scalar.activation(musq[:, :w], mu_ps[:, :w], AF.Square)
        nc.vector.tensor_tensor(self.var_sb[:, t0:t0 + w], sq_ps[:, :w],
                                musq[:, :w], AO.subtract)

    def ln(self):
        nc, tg = self.nc, self.tg
        self.lnv = self.sb1.tile([128, self.L], F32, tag=f"lnv{tg}")
        nc.scalar.activation(self.lnv[:], self.var_sb[:], AF.Ln,
                             bias=self.pk["eps_t"][:])

    def fold(self):
        nc, tg = self.nc, self.tg
        r_bc = self.sb1.tile([128, self.L], BF16, tag=f"rbc{tg}")
        nc.scalar.activation(r_bc[:], self.lnv[:], AF.Exp, scale=-0.5)
        sc_bf = self.wk.tile([128, self.L], BF16, tag=f"sc{tg}")
        nc.vector.tensor_tensor(sc_bf[:], self.hat_bf[:], r_bc[:], AO.mult)
        return sc_bf


def _body(nc, tc, pk):
    """One full forward pass for this core's 4 batches."""
    NJ = pk["NJ"]
    KC = NJ * 128
    quesT2, valsP2_bf, ksc2 = pk["quesT2"], pk["valsP2_bf"], pk["ksc2"]
    wvz_bf = pk["wvz_bf"]
    indz_mu, indz_sq_bf = pk["indz_mu"], pk["indz_sq_bf"]
    glb, eps_t, out_d = pk["glb"], pk["eps_t"], pk["out_d"]

    with tc.tile_pool(name="work", bufs=1) as wk:
        # ================= phase 1: projections + LN folds =================
        with tc.tile_pool(name="ph1sb", bufs=1) as sb1:
            with tc.tile_pool(name="pps", bufs=3, space="PSUM") as projp, \
                 tc.tile_pool(name="mups", bufs=2, space="PSUM") as mup, \
                 tc.tile_pool(name="sqps", bufs=2, space="PSUM") as sqp, \
                 tc.tile_pool(name="musqs", bufs=3) as musqp:
                pools = (projp, mup, sqp, musqp)
                kside = _Ph1Side(nc, pk, wk, sb1, pools, pk["keysT_bf"],
                                 pk["wk_bf"], pk["indmu_k_bf"], KC, "k")
                qside = _Ph1Side(nc, pk, wk, sb1, pools, pk["quesT_bf"],
                                 pk["wq_bf"], pk["indmu_q_bf"], LQ, "q")
                # interleave k/q chunks: PE/ACT/DVE pipeline across the sides
                steps = []
                for i in range(max(len(kside.chunks), len(qside.chunks))):
                    if i < len(kside.chunks):
                        steps.append((kside, kside.chunks[i]))
                    if i < len(qside.chunks):
                        steps.append((qside, qside.chunks[i]))
                for side, t0 in steps:
                    side.step(t0)
            # k side finalizes first so scores can start ASAP; the fold
            # writes the paired block layout directly (strided DVE writes,
            # rows 0:21 only -- guard rows are pre-loaded constants)
            kside.ln()
            k_rbc = sb1.tile([128, KC], BF16, tag="krbc")
            nc.scalar.activation(k_rbc[:], kside.lnv[:], AF.Exp, scale=-0.5)
            for b in range(4):
                rb = 32 * (b % 2) + 64 * (b // 2)
                m = b % 2
                dst = ksc2[rb:rb + 21, :].rearrange(
                    "p (c k) -> p c k", k=128)[:, :, 64 * m:64 * m + 64]
                nc.vector.tensor_tensor(
                    dst,
                    kside.hat_bf[32 * b:32 * b + 21, :].rearrange(
                        "p (c k) -> p c k", k=64),
                    k_rbc[32 * b:32 * b + 21, :].rearrange(
                        "p (c k) -> p c k", k=64),
                    AO.mult)
            qside.ln()
            qsc_bf = qside.fold()
            for b in range(4):
                nc.sync.dma_start(
                    qsc_bf[32 * b + C + 1:32 * b + C + 2, :].bitcast(U16),
                    pk["qrow1_d"][b:b + 1, :])

        if PHASES < 2:
            return

        # ============ phase 2+3: attention + per-tile output LN ============
        with tc.tile_pool(name="scps", bufs=2, space="PSUM") as scps, \
             tc.tile_pool(name="oacc", bufs=2, space="PSUM") as oaccp, \
             tc.tile_pool(name="z1ps", bufs=1, space="PSUM") as z1p, \
             tc.tile_pool(name="muvr", bufs=1, space="PSUM") as muvrp, \
             tc.tile_pool(name="psb", bufs=4) as psb, \
             tc.tile_pool(name="obfp", bufs=4) as obfp, \
             tc.tile_pool(name="ep", bufs=2) as ep:

            o_banks_all = []   # per qt: o_pair (pair A cols 0:NT, B NT:2NT)

            def phase2(qt):
                t0 = qt * NT
                o_pair = oaccp.tile([128, 2 * NT], F32, tag="o")
                o_banks_all.append(o_pair)
                for j in range(NJ):
                    scb = scps.tile([128, 4 * NT], F32, tag="scb")
                    # emission order A1,B1,A2,B2: pair A (rows 0:64) -> bank0
                    # (cols 0:512), pair B (rows 64:128) -> bank1; same-row
                    # matmuls serialize, cross-pair ones hit distinct banks.
                    for pair, h in ((0, 0), (1, 0), (0, 1), (1, 1)):
                        r0 = 64 * pair
                        c0 = 512 * pair + 256 * h
                        nc.tensor.matmul(
                            scb[:, c0:c0 + 256],
                            ksc2[r0:r0 + 64,
                                 128 * (2 * j + h):128 * (2 * j + h) + 128],
                            qsc_bf[r0:r0 + 64, t0:t0 + NT],
                            start=True, stop=True,
                            tile_position=(r0, 0),
                        )
                    # single wide exp over the whole j tile
                    eng = _exp_engine(qt, j)
                    if eng == "A":
                        p_t = psb.tile([128, 4 * NT], BF16, tag="p")
                        nc.scalar.activation(p_t[:], scb[:], AF.Exp,
                                             bias=0.0, scale=float(GS))
                        is_int = False
                    else:
                        p_t = psb.tile([128, 4 * NT], I16, tag="p")
                        nc.vector.tensor_scalar(
                            p_t[:], scb[:],
                            float(GS * A16), float(B16), AO.mult, AO.add)
                        is_int = True
                    # o accumulation: block-diagonal vals/ones stationaries,
                    # mixed-batch key partitions, all at tile_position (0,0)
                    for pair, h in ((0, 0), (1, 0), (0, 1), (1, 1)):
                        c0 = 512 * pair + 256 * h
                        blk = 512 * j + 256 * pair + 128 * h
                        p_sl = p_t[:, c0:c0 + 256]
                        if is_int:
                            p_sl = p_sl.bitcast(BF16)
                        nc.tensor.matmul(
                            o_pair[:, NT * pair:NT * pair + NT],
                            valsP2_bf[:, blk:blk + 128],
                            p_sl,
                            start=(j == 0 and h == 0),
                            stop=(j == NJ - 1 and h == 1),
                            tile_position=(0, 0))

            GRP = 4            # q tiles per Ln/Exp batch (limits table loads)
            d1s = {}           # qt -> d1 tile (alive until its group finalizes)
            var_grp = ep.tile([128, GRP * NT], F32, tag="vgrp")
            var_grp2 = ep.tile([128, GRP * NT], F32, tag="vgrp")
            var_grps = [var_grp, var_grp2]

            def phase3(qt):
                """Everything except the Ln/Exp/zo finalize (see group_fin)."""
                t0 = qt * NT
                o_pair = o_banks_all[qt]
                # o -> SBUF bf16 for the z1 matmul (PE can't read PSUM)
                o_bf0 = obfp.tile([128, NT], BF16, tag="obf")
                nc.scalar.copy(o_bf0[:], o_pair[:, 0:NT])
                o_bf1 = obfp.tile([128, NT], BF16, tag="obf")
                nc.vector.tensor_copy(o_bf1[:], o_pair[:, NT:2 * NT])
                # t = ques * s_bc on Pool: s replicas sit in o_bf rows
                # 32:64 / 96:128; quesT2 holds ques pre-arranged at those
                # partitions (SBUF x SBUF ops need equal base partitions).
                t_sb = ep.tile([128, NT], F32, tag="t")
                for b in range(4):
                    o_bf = o_bf0 if b < 2 else o_bf1
                    r0 = 32 if b % 2 == 0 else 96
                    q2c = t0 if b < 2 else LQ + t0
                    nc.vector.tensor_tensor(
                        t_sb[32 * b:32 * b + 32, :],
                        quesT2[r0:r0 + 32, q2c:q2c + NT],
                        o_bf[r0:r0 + 32, :], AO.mult)
                # z1 = W_v @ o_hat: two accumulating block-diagonal matmuls
                z1_ps = z1p.tile([128, NT], F32, tag="z1")
                nc.tensor.matmul(z1_ps[:], wvz_bf[:, 0:128], o_bf0[:],
                                 start=True, stop=False, tile_position=(0, 0))
                nc.tensor.matmul(z1_ps[:], wvz_bf[:, 128:256], o_bf1[:],
                                 start=False, stop=True, tile_position=(0, 0))
                z_f = ep.tile([128, NT], F32, tag="z")
                nc.vector.tensor_tensor(z_f[:], z1_ps[:], t_sb[:], AO.add)
                # output LN, centered moments, all broadcast via stationaries
                mu_ps = muvrp.tile([128, NT], F32, tag="mv")
                nc.tensor.matmul(mu_ps[:], indz_mu[:], z_f[:],
                                 start=True, stop=True)
                d1 = ep.tile([128, NT], F32, tag=f"d1_{qt % (GRP + 1)}")
                nc.vector.tensor_tensor(d1[:], z_f[:], mu_ps[:], AO.subtract)
                d1s[qt] = d1
                d1sq = ep.tile([128, NT], BF16, tag="d1sq")
                nc.vector.tensor_tensor(d1sq[:], d1[:], d1[:], AO.mult)
                var_ps = muvrp.tile([128, NT], F32, tag="mv")
                nc.tensor.matmul(var_ps[:], indz_sq_bf[:], d1sq[:],
                                 start=True, stop=True)
                # stage var to SBUF (Copy needs no table load) for batched Ln
                g0 = (qt // GRP) * GRP
                nc.scalar.copy(
                    var_grps[(qt // GRP) % 2][:, (qt - g0) * NT:
                                              (qt - g0 + 1) * NT],
                    var_ps[:])

            def group_fin(g0):
                """Batched Ln/Exp + final normalize for q tiles g0..g0+GRP-1."""
                vg = var_grps[(g0 // GRP) % 2]
                lnv = ep.tile([128, GRP * NT], F32, tag="lnv")
                nc.scalar.activation(lnv[:], vg[:], AF.Ln, bias=eps_t[:])
                rstdg = ep.tile([128, GRP * NT], F32, tag="rstdg")
                nc.scalar.activation(rstdg[:], lnv[:], AF.Exp, scale=-0.5,
                                     bias=glb[:, 2:3])
                for qt in range(g0, g0 + GRP):
                    c0 = (qt - g0) * NT
                    zo = ep.tile([128, NT], F32, tag="zo")
                    nc.vector.tensor_tensor(zo[:], d1s[qt][:],
                                            rstdg[:, c0:c0 + NT], AO.mult)
                    nc.sync.dma_start(out_d[:, qt * NT:qt * NT + NT], zo[:])

            for qt in range(NQT):
                phase2(qt)
                if PHASES >= 3:
                    if qt > 0:
                        phase3(qt - 1)
                    if qt % GRP == 0 and qt >= GRP:
                        group_fin(qt - GRP)
            if PHASES >= 3:
                phase3(NQT - 1)
                group_fin(NQT - GRP)


# ---------------------------------------------------------------------------
# host side
# ---------------------------------------------------------------------------

def _bf16_bits(x):
    u = np.ascontiguousarray(x, np.float32).view(np.uint32)
    return ((u + 0x7FFF + ((u >> 16) & 1)) >> 16).astype(np.uint16)


def prepare_inputs(vals, keys, ques, key_mask, W_v, W_k, W_q,
                   g_k, b_k, g_q, b_q, g_o, b_o):
    """Shard + lay out the full inputs for the 8 cores. Returns (in_maps, KC)."""
    vals = np.ascontiguousarray(vals, np.float32)
    keys = np.ascontiguousarray(keys, np.float32)
    ques = np.ascontiguousarray(ques, np.float32)
    key_mask = np.asarray(key_mask)
    W_v = np.asarray(W_v, np.float32)
    W_k = np.asarray(W_k, np.float32)
    W_q = np.asarray(W_q, np.float32)
    g_k = np.asarray(g_k, np.float32)
    b_k = np.asarray(b_k, np.float32)
    g_q = np.asarray(g_q, np.float32)
    b_q = np.asarray(b_q, np.float32)
    g_o = np.asarray(g_o, np.float32)
    b_o = np.asarray(b_o, np.float32)

    # supported parameterization (holds for the harness inputs)
    if not (np.allclose(b_k, 0) and np.allclose(b_q, 0)):
        raise NotImplementedError("nonzero k/q LN bias not supported")
    if not (np.allclose(g_k, g_k.flat[0]) and np.allclose(g_q, g_q.flat[0])):
        raise NotImplementedError("non-uniform k/q LN gain not supported")
    if not (np.allclose(b_o, 0) and np.all(g_o > 0)):
        raise NotImplementedError("output LN with b_o!=0 or g_o<=0")
    guni = float(g_k.flat[0] * g_q.flat[0])

    counts = (~key_mask).sum(axis=1)
    KC = int(np.ceil(max(int(counts.max()), 1) / 128) * 128)
    NJ = KC // 128

    wq_aug = np.zeros((32, 32), np.float32)
    wq_aug[:, :C] = W_q.T
    wq_aug[:, C] = W_q.sum(axis=0) / S20
    wq_aug *= guni     # fold uniform LN gains into the q side
    wk_aug = np.zeros((32, 32), np.float32)
    wk_aug[:, :C] = W_k.T
    wk_aug[:, C] = -W_k.sum(axis=0) / S20

    wq_st = np.zeros((128, 32), np.float32)
    wk_st = np.zeros((128, 32), np.float32)
    wvz = np.zeros((128, 256), np.float32)
    # block-diagonal W_v for the merged z1 matmuls: o_bf0 rows (b0 dims @0:32,
    # b1 dims @64:96) -> z1 rows 0:32 / 32:64; o_bf1 -> z1 rows 64:96 / 96:128
    wvz[0:32, 0:32] = W_v.T
    wvz[64:96, 32:64] = W_v.T
    wvz[0:32, 128 + 64:128 + 96] = W_v.T
    wvz[64:96, 128 + 96:128 + 128] = W_v.T
    indmu_q = np.zeros((128, 128), np.float32)
    indmu_k = np.zeros((128, 128), np.float32)
    indsq = np.zeros((128, 128), np.float32)
    indz_mu = np.zeros((128, 128), np.float32)
    indz_sq = np.zeros((128, 128), np.float32)
    glb = np.zeros((128, 4), np.float32)
    for b in range(BPC):
        r = 32 * b
        wq_st[r:r + 32] = wq_aug
        wk_st[r:r + 32] = wk_aug
        for c in range(CAUG + 1):
            indmu_q[r + C, r + c] = S20 / C
            indmu_k[r + C, r + c] = -S20 / C
            indsq[r:r + C, r + c] = 1.0 / C
        indz_mu[r:r + 32, r:r + 32] = 1.0 / D
        indz_sq[r:r + 32, r:r + 32] = 1.0 / D
        glb[r:r + 32, 0] = g_o
        glb[r:r + 32, 1] = b_o
        glb[r:r + 32, 2] = np.log(g_o)

    in_maps = []
    for cid in range(NCORES):
        quesT = np.zeros((128, LQ), np.uint16)
        quesT2 = np.zeros((128, 2 * LQ), np.uint16)
        keysT = np.zeros((128, KC), np.uint16)
        valsP2 = np.zeros((128, NJ * 512), np.uint16)
        krow1 = np.zeros((4, KC), np.uint16)
        one_bits = np.uint16(0x3F80)
        for b in range(BPC):
            g = cid * BPC + b
            idx = np.flatnonzero(~key_mask[g])
            ci = len(idx)
            qb = _bf16_bits(ques[g].T)
            quesT[32 * b:32 * b + 32] = qb
            r0 = 32 if b % 2 == 0 else 96
            c0 = 0 if b < 2 else LQ
            quesT2[r0:r0 + 32, c0:c0 + LQ] = qb
            keysT[32 * b:32 * b + 32, :ci] = _bf16_bits(keys[g][idx].T)
            vc = np.zeros((KC, D), np.float32)
            vc[:ci] = vals[g][idx]
            vcb = _bf16_bits(vc)
            pair, half = b // 2, b % 2
            for j in range(NJ):
                for h in range(2):
                    blk = 512 * j + 256 * pair + 128 * h
                    rows = slice(64 * half, 64 * half + 64)
                    cols = slice(blk + 64 * half, blk + 64 * half + 32)
                    valsP2[rows, cols] = vcb[128 * j + 64 * h:
                                             128 * j + 64 * h + 64]
                    valsP2[rows, cols.stop:cols.stop + 32] = one_bits
            krow1[b, ci:] = np.float32(-300.0).view(np.uint32) >> 16
        in_maps.append({
            "quesT": quesT, "quesT2": quesT2, "keysT": keysT,
            "valsP2": valsP2,
            "wq_st": wq_st, "wk_st": wk_st, "wvz": wvz,
            "indmu_q": indmu_q, "indmu_k": indmu_k, "indsq": indsq,
            "indz_mu": indz_mu, "indz_sq": indz_sq, "glb": glb,
            "qrow1": np.full((4, LQ), 0x3F80, np.uint16),
            "krow1": krow1,
        })
    return in_maps, KC


def unshard_output(results):
    out = np.empty((B, LQ, D), np.float32)
    for cid in range(NCORES):
        o = results[cid]["out"]
        for b in range(BPC):
            out[cid * BPC + b] = o[32 * b:32 * b + 32, :].T
    return out


def kernel(**inputs) -> np.ndarray:
    in_maps, KC = prepare_inputs(**inputs)
    key = ("nc", KC)
    if key not in _cache:
        _cache[key] = build_module(KC)
    nc = _cache[key]
    res = bass_utils.run_bass_kernel_spmd(nc, in_maps,
                                          core_ids=list(range(NCORES)))
    return unshard_output(res.results)


# revision 3
# speedup vs baseline: 1.0065x; 1.0065x over previous
"""TRN2 Bass kernel v2 for nn_Attention_21758304322201 (sparse_attention).

Reference computation (B=32, L=2048, D=32, C=20):
    v = vals @ W_v.T
    k = LN(keys @ W_k.T);  q = LN(ques @ W_q.T)
    a = q @ k.T / sqrt(C);  a[masked keys] = -inf
    p = softmax(a);  o = p @ v
    out = LN(o + ques)

v2 restructure (vs the v1 baseline) — targets the dependency stalls that
dominated v1's trace (engines <35% busy on real work):
  * Batches are processed in PAIRS with block-diagonal stationaries: the
    score matmuls contract 64 rows (two 22-dim batch blocks) and cover half
    a key chunk of two batches at once; the o matmuls contract a mixed
    128-key partition layout with a block-diagonal vals/ones stationary.
    Safety: concurrent row-disjoint matmuls writing the SAME PSUM bank fault
    the HW (verified empirically) — here pair A (PE rows 0:64) targets bank
    0 and pair B (rows 64:128) bank 1; same-row matmuls serialize in HW.
  * One exp instruction per j over a [128, 1024] 2-bank scores tile
    (all 4 batches), rotated between ACT (native exp) and DVE (int16
    Schraudolph bf16-bit trick). Pool/GpSimd cannot read PSUM.
  * All row-stat broadcasts (LN rstd of q/k, output-LN mu/var) are produced
    pre-broadcast by 128-column indicator stationaries in the stats matmuls
    (PE stream cost depends only on N) -> no DRAM bounce, no broadcast DMAs.
  * The softmax normalizer rows are replicated 32x for free by ones-column
    blocks in the vals stationary; the s*q residual term reads them via a
    pre-arranged bf16 ques layout (quesT2). All elementwise work runs on
    ACT/DVE: the Pool/GpSimd engine measured ~3x slower end to end (poor Q7
    software pipelining + SBUF-port contention with the DVE).
  * Output LN is per-q-tile with centered moments (d1 = z - mu_bc,
    var = E[d1^2]); Ln/Exp are batched across groups of 4 q tiles to avoid
    ACT table-set thrash; g_o folds into the Exp bias (ln g).
  * ques/keys/vals ship as bf16 bits from the host (no on-device f32->bf16
    conversions; projection/stat matmuls stream bf16 anyway).
"""
import math

import numpy as np

from concourse import bacc, bass, bass_utils, tile
from concourse import mybir

dt = mybir.dt
F32 = dt.float32
BF16 = dt.bfloat16
I16 = dt.int16
U16 = dt.uint16
AO = mybir.AluOpType
AF = mybir.ActivationFunctionType

# problem constants (hardcoded per harness contract)
B, LQ, LK, D, C = 32, 2048, 2048, 32, 20
EPS = 1e-5
NCORES = 8
BPC = B // NCORES          # batches per core = 4
CAUG = C + 1               # 21-dim augmented projection (mean fold)
CDIM = C + 2               # +1 guard dim for pad masking
NT = 256                   # q-tile width
NQT = LQ // NT             # 8 q tiles
S20 = math.sqrt(C)
GS = 1.0 / S20             # score scale

# int16 Schraudolph (bf16 bit pattern): bits = round(x * A16 + B16)
A16 = 128.0 / math.log(2.0)
B16 = 127.0 * 128.0 - 5.6          # max rel err ~3.3%, mean ~1.8%

# exp engine rotation per j (one [128,1024] exp per j): 'A' = ACT, 'D' = DVE.
# STRICT alternation across j (including qt boundaries): adjacent j's must
# exp on different engines or the 2-deep scores pipeline serializes.
EXP_PATTERN = ("ADADADADA", "DADADADAD")

PHASES = 3
_cache: dict = {}


def _exp_engine(qt, j):
    pat = EXP_PATTERN[qt % len(EXP_PATTERN)]
    return pat[j % len(pat)]




def build_module(KC: int, reps: int = 1):
    """Build the SPMD bass module for per-core work. KC = padded key count."""
    NJ = KC // 128
    nc = bacc.Bacc("TRN2", target_bir_lowering=False, debug=False,
                   num_devices=NCORES)

    def din(name, shape, dty=F32):
        return nc.dram_tensor(name, shape, dty, kind="ExternalInput").ap()

    quesT_d = din("quesT", [128, LQ], U16)
    quesT2_d = din("quesT2", [128, 2 * LQ], U16)
    keysT_d = din("keysT", [128, KC], U16)
    valsP2_d = din("valsP2", [128, NJ * 512], U16)
    wq_d = din("wq_st", [128, 32])
    wk_d = din("wk_st", [128, 32])
    wvz_d = din("wvz", [128, 256])
    indmu_q_d = din("indmu_q", [128, 128])
    indmu_k_d = din("indmu_k", [128, 128])
    indsq_d = din("indsq", [128, 128])
    indz_mu_d = din("indz_mu", [128, 128])
    indz_sq_d = din("indz_sq", [128, 128])
    glb_d = din("glb", [128, 4])     # cols: g_o, b_o, ln(g_o), (pad)
    qrow1_d = nc.dram_tensor("qrow1", [4, LQ], U16, kind="ExternalInput").ap()
    krow1_d = nc.dram_tensor("krow1", [4, KC], U16, kind="ExternalInput").ap()
    out_d = nc.dram_tensor("out", [128, LQ], F32, kind="ExternalOutput").ap()

    with tile.TileContext(nc) as tc:
        with tc.tile_pool(name="inp", bufs=1) as inp, \
             tc.tile_pool(name="cst", bufs=1) as cst:
            # ---- load inputs (already bf16 where applicable) ----
            quesT_bf = inp.tile([128, LQ], BF16)
            nc.sync.dma_start(quesT_bf[:].bitcast(U16), quesT_d)
            quesT2 = inp.tile([128, 2 * LQ], BF16)
            nc.sync.dma_start(quesT2[:].bitcast(U16), quesT2_d)
            keysT_bf = inp.tile([128, KC], BF16)
            nc.sync.dma_start(keysT_bf[:].bitcast(U16), keysT_d)
            valsP2_bf = inp.tile([128, NJ * 512], BF16)
            nc.sync.dma_start(valsP2_bf[:].bitcast(U16), valsP2_d)
            wq_f = cst.tile([128, 32], F32)
            nc.sync.dma_start(wq_f[:], wq_d)
            wk_f = cst.tile([128, 32], F32)
            nc.sync.dma_start(wk_f[:], wk_d)
            wvz_f = cst.tile([128, 256], F32)
            nc.sync.dma_start(wvz_f[:], wvz_d)
            indmu_q_f = cst.tile([128, 128], F32)
            nc.sync.dma_start(indmu_q_f[:], indmu_q_d)
            indmu_k_f = cst.tile([128, 128], F32)
            nc.sync.dma_start(indmu_k_f[:], indmu_k_d)
            indsq_f = cst.tile([128, 128], F32)
            nc.sync.dma_start(indsq_f[:], indsq_d)
            indz_mu = cst.tile([128, 128], F32)
            nc.sync.dma_start(indz_mu[:], indz_mu_d)
            indz_sq_f = cst.tile([128, 128], F32)
            nc.sync.dma_start(indz_sq_f[:], indz_sq_d)
            glb = cst.tile([128, 4], F32)
            nc.sync.dma_start(glb[:], glb_d)

            # ---- one-time constant prep ----
            wq_bf = cst.tile([128, 32], BF16)
            nc.vector.tensor_copy(wq_bf[:], wq_f[:])
            wk_bf = cst.tile([128, 32], BF16)
            nc.vector.tensor_copy(wk_bf[:], wk_f[:])
            wvz_bf = cst.tile([128, 256], BF16)
            nc.vector.tensor_copy(wvz_bf[:], wvz_f[:])
            indmu_q_bf = cst.tile([128, 128], BF16)
            nc.vector.tensor_copy(indmu_q_bf[:], indmu_q_f[:])
            indmu_k_bf = cst.tile([128, 128], BF16)
            nc.vector.tensor_copy(indmu_k_bf[:], indmu_k_f[:])
            indsq_bf = cst.tile([128, 128], BF16)
            nc.vector.tensor_copy(indsq_bf[:], indsq_f[:])
            indz_sq_bf = cst.tile([128, 128], BF16)
            nc.vector.tensor_copy(indz_sq_bf[:], indz_sq_f[:])
            eps_t = cst.tile([128, 1], F32)
            nc.gpsimd.memset(eps_t[:], EPS)
            # paired-layout k stationary; zero blocks + guard rows persist
            # across reps (per-rep folds only write rows rb+0:21)
            ksc2 = cst.tile([128, 2 * KC], BF16)
            nc.gpsimd.memset(ksc2[:], 0.0)
            for b in range(4):
                rb = 32 * (b % 2) + 64 * (b // 2)
                m = b % 2
                gdst = ksc2[rb + 21:rb + 22, :].rearrange(
                    "p (c k) -> p c k", k=128)[:, :, 64 * m:64 * m + 64]
                nc.sync.dma_start(
                    gdst.bitcast(U16),
                    krow1_d[b:b + 1, :].rearrange("p (c k) -> p c k", k=64))

            pk = dict(
                NJ=NJ, quesT_bf=quesT_bf, quesT2=quesT2, keysT_bf=keysT_bf,
                valsP2_bf=valsP2_bf, ksc2=ksc2,
                wq_bf=wq_bf, wk_bf=wk_bf, wvz_bf=wvz_bf,
                indmu_q_bf=indmu_q_bf, indmu_k_bf=indmu_k_bf,
                indsq_bf=indsq_bf, indz_mu=indz_mu, indz_sq_bf=indz_sq_bf,
                glb=glb, eps_t=eps_t, out_d=out_d,
                qrow1_d=qrow1_d, krow1_d=krow1_d,
            )

            if reps == 1:
                _body(nc, tc, pk)
            elif reps == 0:
                pass
            else:
                with tc.For_i(0, reps, 1):
                    _body(nc, tc, pk)

    nc.compile()
    return nc


class _Ph1Side:
    """One side (q or k) of phase 1, chunk-steppable for k/q interleaving."""

    def __init__(self, nc, pk, wk, sb1, pools, src_bf, w_bf, indmu_bf, L, tg):
        self.nc, self.pk, self.L, self.tg = nc, pk, L, tg
        self.src_bf, self.w_bf, self.indmu_bf = src_bf, w_bf, indmu_bf
        self.hat_bf = wk.tile([128, L], BF16, tag=f"hat{tg}")
        self.sq_bf = sb1.tile([128, L], BF16, tag=f"sq{tg}")
        self.var_sb = sb1.tile([128, L], F32, tag=f"var{tg}")
        self.sb1 = sb1
        self.wk = wk
        self.pools = pools       # shared (projp, mup, sqp, musqp)
        self.chunks = list(range(0, L, 512))

    def step(self, t0):
        nc = self.nc
        w = min(512, self.L - t0)
        projp, mup, sqp, musqp = self.pools
        proj_ps = projp.tile([128, 512], F32, tag="pp")
        for b in range(4):
            nc.tensor.matmul(
                proj_ps[32 * b:32 * b + 32, :w],
                self.w_bf[32 * b:32 * b + 32, :],
                self.src_bf[32 * b:32 * b + 32, t0:t0 + w],
                start=True, stop=True,
                tile_position=(32 * b, 32 * b),
            )
        # PSUM -> SBUF bf16 (ACT) + elementwise square (DVE)
        nc.scalar.copy(self.hat_bf[:, t0:t0 + w], proj_ps[:, :w])
        nc.vector.tensor_tensor(self.sq_bf[:, t0:t0 + w],
                                self.hat_bf[:, t0:t0 + w],
                                self.hat_bf[:, t0:t0 + w], AO.mult)
        mu_ps = mup.tile([128, 512], F32, tag="mu")
        nc.tensor.matmul(mu_ps[:, :w], self.indmu_bf[:],
                         self.hat_bf[:, t0:t0 + w], start=True, stop=True)
        sq_ps = sqp.tile([128, 512], F32, tag="sq")
        nc.tensor.matmul(sq_ps[:, :w], self.pk["indsq_bf"][:],
                         self.sq_bf[:, t0:t0 + w], start=True, stop=True)
        musq = musqp.tile([128, 512], F32, tag="musq")
        nc.scalar.activation(musq[:, :w], mu_ps[:, :w], AF.Square)
        nc.vector.tensor_tensor(self.var_sb[:, t0:t0 + w], sq_ps[:, :w],
                                musq[:, :w], AO.subtract)

    def ln(self):
        nc, tg = self.nc, self.tg
        self.lnv = self.sb1.tile([128, self.L], F32, tag=f"lnv{tg}")
        nc.scalar.activation(self.lnv[:], self.var_sb[:], AF.Ln,
                             bias=self.pk["eps_t"][:])

    def fold(self):
        nc, tg = self.nc, self.tg
        r_bc = self.sb1.tile([128, self.L], BF16, tag=f"rbc{tg}")
        nc.scalar.activation(r_bc[:], self.lnv[:], AF.Exp, scale=-0.5)
        sc_bf = self.wk.tile([128, self.L], BF16, tag=f"sc{tg}")
        nc.vector.tensor_tensor(sc_bf[:], self.hat_bf[:], r_bc[:], AO.mult)
        return sc_bf


def _body(nc, tc, pk):
    """One full forward pass for this core's 4 batches."""
    NJ = pk["NJ"]
    KC = NJ * 128
    quesT2, valsP2_bf, ksc2 = pk["quesT2"], pk["valsP2_bf"], pk["ksc2"]
    wvz_bf = pk["wvz_bf"]
    indz_mu, indz_sq_bf = pk["indz_mu"], pk["indz_sq_bf"]
    glb, eps_t, out_d = pk["glb"], pk["eps_t"], pk["out_d"]

    with tc.tile_pool(name="work", bufs=1) as wk:
        # ================= phase 1: projections + LN folds =================
        with tc.tile_pool(name="ph1sb", bufs=1) as sb1:
            with tc.tile_pool(name="pps", bufs=3, space="PSUM") as projp, \
                 tc.tile_pool(name="mups", bufs=2, space="PSUM") as mup, \
                 tc.tile_pool(name="sqps", bufs=2, space="PSUM") as sqp, \
                 tc.tile_pool(name="musqs", bufs=3) as musqp:
                pools = (projp, mup, sqp, musqp)
                kside = _Ph1Side(nc, pk, wk, sb1, pools, pk["keysT_bf"],
                                 pk["wk_bf"], pk["indmu_k_bf"], KC, "k")
                qside = _Ph1Side(nc, pk, wk, sb1, pools, pk["quesT_bf"],
                                 pk["wq_bf"], pk["indmu_q_bf"], LQ, "q")
                # interleave k/q chunks: PE/ACT/DVE pipeline across the sides
                steps = []
                for i in range(max(len(kside.chunks), len(qside.chunks))):
                    if i < len(kside.chunks):
                        steps.append((kside, kside.chunks[i]))
                    if i < len(qside.chunks):
                        steps.append((qside, qside.chunks[i]))
                for side, t0 in steps:
                    side.step(t0)
            # k side finalizes first so scores can start ASAP; the fold
            # writes the paired block layout directly (strided DVE writes,
            # rows 0:21 only -- guard rows are pre-loaded constants)
            kside.ln()
            k_rbc = sb1.tile([128, KC], BF16, tag="krbc")
            nc.scalar.activation(k_rbc[:], kside.lnv[:], AF.Exp, scale=-0.5)
            for b in range(4):
                rb = 32 * (b % 2) + 64 * (b // 2)
                m = b % 2
                dst = ksc2[rb:rb + 21, :].rearrange(
                    "p (c k) -> p c k", k=128)[:, :, 64 * m:64 * m + 64]
                nc.vector.tensor_tensor(
                    dst,
                    kside.hat_bf[32 * b:32 * b + 21, :].rearrange(
                        "p (c k) -> p c k", k=64),
                    k_rbc[32 * b:32 * b + 21, :].rearrange(
                        "p (c k) -> p c k", k=64),
                    AO.mult)
            qside.ln()
            qsc_bf = qside.fold()
            for b in range(4):
                nc.sync.dma_start(
                    qsc_bf[32 * b + C + 1:32 * b + C + 2, :].bitcast(U16),
                    pk["qrow1_d"][b:b + 1, :])

        if PHASES < 2:
            return

        # ============ phase 2+3: attention + per-tile output LN ============
        with tc.tile_pool(name="scps", bufs=2, space="PSUM") as scps, \
             tc.tile_pool(name="oacc", bufs=2, space="PSUM") as oaccp, \
             tc.tile_pool(name="z1ps", bufs=1, space="PSUM") as z1p, \
             tc.tile_pool(name="muvr", bufs=1, space="PSUM") as muvrp, \
             tc.tile_pool(name="psb", bufs=4) as psb, \
             tc.tile_pool(name="obfp", bufs=4) as obfp, \
             tc.tile_pool(name="ep", bufs=2) as ep:

            o_banks_all = []   # per qt: o_pair (pair A cols 0:NT, B NT:2NT)

            def phase2(qt):
                t0 = qt * NT
                o_pair = oaccp.tile([128, 2 * NT], F32, tag="o")
                o_banks_all.append(o_pair)
                for j in range(NJ):
                    scb = scps.tile([128, 4 * NT], F32, tag="scb")
                    # emission order A1,B1,A2,B2: pair A (rows 0:64) -> bank0
                    # (cols 0:512), pair B (rows 64:128) -> bank1; same-row
                    # matmuls serialize, cross-pair ones hit distinct banks.
                    for pair, h in ((0, 0), (1, 0), (0, 1), (1, 1)):
                        r0 = 64 * pair
                        c0 = 512 * pair + 256 * h
                        nc.tensor.matmul(
                            scb[:, c0:c0 + 256],
                            ksc2[r0:r0 + 64,
                                 128 * (2 * j + h):128 * (2 * j + h) + 128],
                            qsc_bf[r0:r0 + 64, t0:t0 + NT],
                            start=True, stop=True,
                            tile_position=(r0, 0),
                        )
                    # single wide exp over the whole j tile
                    eng = _exp_engine(qt, j)
                    if eng == "A":
                        p_t = psb.tile([128, 4 * NT], BF16, tag="p")
                        nc.scalar.activation(p_t[:], scb[:], AF.Exp,
                                             bias=0.0, scale=float(GS))
                        is_int = False
                    else:
                        p_t = psb.tile([128, 4 * NT], I16, tag="p")
                        nc.vector.tensor_scalar(
                            p_t[:], scb[:],
                            float(GS * A16), float(B16), AO.mult, AO.add)
                        is_int = True
                    # o accumulation: block-diagonal vals/ones stationaries,
                    # mixed-batch key partitions, all at tile_position (0,0)
                    for pair, h in ((0, 0), (1, 0), (0, 1), (1, 1)):
                        c0 = 512 * pair + 256 * h
                        blk = 512 * j + 256 * pair + 128 * h
                        p_sl = p_t[:, c0:c0 + 256]
                        if is_int:
                            p_sl = p_sl.bitcast(BF16)
                        nc.tensor.matmul(
                            o_pair[:, NT * pair:NT * pair + NT],
                            valsP2_bf[:, blk:blk + 128],
                            p_sl,
                            start=(j == 0 and h == 0),
                            stop=(j == NJ - 1 and h == 1),
                            tile_position=(0, 0))

            GRP = 4            # q tiles per Ln/Exp batch (limits table loads)
            d1s = {}           # qt -> d1 tile (alive until its group finalizes)
            var_grp = ep.tile([128, GRP * NT], F32, tag="vgrp")
            var_grp2 = ep.tile([128, GRP * NT], F32, tag="vgrp")
            var_grps = [var_grp, var_grp2]

            def phase3(qt):
                """Everything except the Ln/Exp/zo finalize (see group_fin)."""
                t0 = qt * NT
                o_pair = o_banks_all[qt]
                # o -> SBUF bf16 for the z1 matmul (PE can't read PSUM)
                o_bf0 = obfp.tile([128, NT], BF16, tag="obf")
                nc.scalar.copy(o_bf0[:], o_pair[:, 0:NT])
                o_bf1 = obfp.tile([128, NT], BF16, tag="obf")
                nc.vector.tensor_copy(o_bf1[:], o_pair[:, NT:2 * NT])
                # t = ques * s_bc on Pool: s replicas sit in o_bf rows
                # 32:64 / 96:128; quesT2 holds ques pre-arranged at those
                # partitions (SBUF x SBUF ops need equal base partitions).
                t_sb = ep.tile([128, NT], F32, tag="t")
                for b in range(4):
                    o_bf = o_bf0 if b < 2 else o_bf1
                    r0 = 32 if b % 2 == 0 else 96
                    q2c = t0 if b < 2 else LQ + t0
                    nc.vector.tensor_tensor(
                        t_sb[32 * b:32 * b + 32, :],
                        quesT2[r0:r0 + 32, q2c:q2c + NT],
                        o_bf[r0:r0 + 32, :], AO.mult)
                # z1 = W_v @ o_hat: two accumulating block-diagonal matmuls
                z1_ps = z1p.tile([128, NT], F32, tag="z1")
                nc.tensor.matmul(z1_ps[:], wvz_bf[:, 0:128], o_bf0[:],
                                 start=True, stop=False, tile_position=(0, 0))
                nc.tensor.matmul(z1_ps[:], wvz_bf[:, 128:256], o_bf1[:],
                                 start=False, stop=True, tile_position=(0, 0))
                z_f = ep.tile([128, NT], F32, tag="z")
                nc.vector.tensor_tensor(z_f[:], z1_ps[:], t_sb[:], AO.add)
                # output LN, centered moments, all broadcast via stationaries
                mu_ps = muvrp.tile([128, NT], F32, tag="mv")
                nc.tensor.matmul(mu_ps[:], indz_mu[:], z_f[:],
                                 start=True, stop=True)
                d1 = ep.tile([128, NT], F32, tag=f"d1_{qt % (GRP + 1)}")
                nc.vector.tensor_tensor(d1[:], z_f[:], mu_ps[:], AO.subtract)
                d1s[qt] = d1
                d1sq = ep.tile([128, NT], BF16, tag="d1sq")
                nc.vector.tensor_tensor(d1sq[:], d1[:], d1[:], AO.mult)
                var_ps = muvrp.tile([128, NT], F32, tag="mv")
                nc.tensor.matmul(var_ps[:], indz_sq_bf[:], d1sq[:],
                                 start=True, stop=True)
                # stage var to SBUF (Copy needs no table load) for batched Ln
                g0 = (qt // GRP) * GRP
                nc.scalar.copy(
                    var_grps[(qt // GRP) % 2][:, (qt - g0) * NT:
                                              (qt - g0 + 1) * NT],
                    var_ps[:])

            def group_fin(g0):
                """Batched Ln/Exp + final normalize for q tiles g0..g0+GRP-1."""
                vg = var_grps[(g0 // GRP) % 2]
                lnv = ep.tile([128, GRP * NT], F32, tag="lnv")
                nc.scalar.activation(lnv[:], vg[:], AF.Ln, bias=eps_t[:])
                rstdg = ep.tile([128, GRP * NT], F32, tag="rstdg")
                nc.scalar.activation(rstdg[:], lnv[:], AF.Exp, scale=-0.5,
                                     bias=glb[:, 2:3])
                for qt in range(g0, g0 + GRP):
                    c0 = (qt - g0) * NT
                    zo = ep.tile([128, NT], F32, tag="zo")
                    nc.vector.tensor_tensor(zo[:], d1s[qt][:],
                                            rstdg[:, c0:c0 + NT], AO.mult)
                    nc.sync.dma_start(out_d[:, qt * NT:qt * NT + NT], zo[:])

            for qt in range(NQT):
                phase2(qt)
                if PHASES >= 3:
                    if qt > 0:
                        phase3(qt - 1)
                    if qt % GRP == 0 and qt >= GRP:
                        group_fin(qt - GRP)
            if PHASES >= 3:
                phase3(NQT - 1)
                group_fin(NQT - GRP)


# ---------------------------------------------------------------------------
# host side
# ---------------------------------------------------------------------------

def _bf16_bits(x):
    u = np.ascontiguousarray(x, np.float32).view(np.uint32)
    return ((u + 0x7FFF + ((u >> 16) & 1)) >> 16).astype(np.uint16)


def prepare_inputs(vals, keys, ques, key_mask, W_v, W_k, W_q,
                   g_k, b_k, g_q, b_q, g_o, b_o):
    """Shard + lay out the full inputs for the 8 cores. Returns (in_maps, KC)."""
    vals = np.ascontiguousarray(vals, np.float32)
    keys = np.ascontiguousarray(keys, np.float32)
    ques = np.ascontiguousarray(ques, np.float32)
    key_mask = np.asarray(key_mask)
    W_v = np.asarray(W_v, np.float32)
    W_k = np.asarray(W_k, np.float32)
    W_q = np.asarray(W_q, np.float32)
    g_k = np.asarray(g_k, np.float32)
    b_k = np.asarray(b_k, np.float32)
    g_q = np.asarray(g_q, np.float32)
    b_q = np.asarray(b_q, np.float32)
    g_o = np.asarray(g_o, np.float32)
    b_o = np.asarray(b_o, np.float32)

    # supported parameterization (holds for the harness inputs)
    if not (np.allclose(b_k, 0) and np.allclose(b_q, 0)):
        raise NotImplementedError("nonzero k/q LN bias not supported")
    if not (np.allclose(g_k, g_k.flat[0]) and np.allclose(g_q, g_q.flat[0])):
        raise NotImplementedError("non-uniform k/q LN gain not supported")
    if not (np.allclose(b_o, 0) and np.all(g_o > 0)):
        raise NotImplementedError("output LN with b_o!=0 or g_o<=0")
    guni = float(g_k.flat[0] * g_q.flat[0])

    counts = (~key_mask).sum(axis=1)
    KC = int(np.ceil(max(int(counts.max()), 1) / 128) * 128)
    NJ = KC // 128

    wq_aug = np.zeros((32, 32), np.float32)
    wq_aug[:, :C] = W_q.T
    wq_aug[:, C] = W_q.sum(axis=0) / S20
    wq_aug *= guni     # fold uniform LN gains into the q side
    wk_aug = np.zeros((32, 32), np.float32)
    wk_aug[:, :C] = W_k.T
    wk_aug[:, C] = -W_k.sum(axis=0) / S20

    wq_st = np.zeros((128, 32), np.float32)
    wk_st = np.zeros((128, 32), np.float32)
    wvz = np.zeros((128, 256), np.float32)
    # block-diagonal W_v for the merged z1 matmuls: o_bf0 rows (b0 dims @0:32,
    # b1 dims @64:96) -> z1 rows 0:32 / 32:64; o_bf1 -> z1 rows 64:96 / 96:128
    wvz[0:32, 0:32] = W_v.T
    wvz[64:96, 32:64] = W_v.T
    wvz[0:32, 128 + 64:128 + 96] = W_v.T
    wvz[64:96, 128 + 96:128 + 128] = W_v.T
    indmu_q = np.zeros((128, 128), np.float32)
    indmu_k = np.zeros((128, 128), np.float32)
    indsq = np.zeros((128, 128), np.float32)
    indz_mu = np.zeros((128, 128), np.float32)
    indz_sq = np.zeros((128, 128), np.float32)
    glb = np.zeros((128, 4), np.float32)
    for b in range(BPC):
        r = 32 * b
        wq_st[r:r + 32] = wq_aug
        wk_st[r:r + 32] = wk_aug
        for c in range(CAUG + 1):
            indmu_q[r + C, r + c] = S20 / C
            indmu_k[r + C, r + c] = -S20 / C
            indsq[r:r + C, r + c] = 1.0 / C
        indz_mu[r:r + 32, r:r + 32] = 1.0 / D
        indz_sq[r:r + 32, r:r + 32] = 1.0 / D
        glb[r:r + 32, 0] = g_o
        glb[r:r + 32, 1] = b_o
        glb[r:r + 32, 2] = np.log(g_o)

    in_maps = []
    for cid in range(NCORES):
        quesT = np.zeros((128, LQ), np.uint16)
        quesT2 = np.zeros((128, 2 * LQ), np.uint16)
        keysT = np.zeros((128, KC), np.uint16)
        valsP2 = np.zeros((128, NJ * 512), np.uint16)
        krow1 = np.zeros((4, KC), np.uint16)
        one_bits = np.uint16(0x3F80)
        for b in range(BPC):
            g = cid * BPC + b
            idx = np.flatnonzero(~key_mask[g])
            ci = len(idx)
            qb = _bf16_bits(ques[g].T)
            quesT[32 * b:32 * b + 32] = qb
            r0 = 32 if b % 2 == 0 else 96
            c0 = 0 if b < 2 else LQ
            quesT2[r0:r0 + 32, c0:c0 + LQ] = qb
            keysT[32 * b:32 * b + 32, :ci] = _bf16_bits(keys[g][idx].T)
            vc = np.zeros((KC, D), np.float32)
            vc[:ci] = vals[g][idx]
            vcb = _bf16_bits(vc)
            pair, half = b // 2, b % 2
            for j in range(NJ):
                for h in range(2):
                    blk = 512 * j + 256 * pair + 128 * h
                    rows = slice(64 * half, 64 * half + 64)
                    cols = slice(blk + 64 * half, blk + 64 * half + 32)
                    valsP2[rows, cols] = vcb[128 * j + 64 * h:
                                             128 * j + 64 * h + 64]
                    valsP2[rows, cols.stop:cols.stop + 32] = one_bits
            krow1[b, ci:] = np.float32(-300.0).view(np.uint32) >> 16
        in_maps.append({
            "quesT": quesT, "quesT2": quesT2, "keysT": keysT,
            "valsP2": valsP2,
            "wq_st": wq_st, "wk_st": wk_st, "wvz": wvz,
            "indmu_q": indmu_q, "indmu_k": indmu_k, "indsq": indsq,
            "indz_mu": indz_mu, "indz_sq": indz_sq, "glb": glb,
            "qrow1": np.full((4, LQ), 0x3F80, np.uint16),
            "krow1": krow1,
        })
    return in_maps, KC


def unshard_output(results):
    out = np.empty((B, LQ, D), np.float32)
    for cid in range(NCORES):
        o = results[cid]["out"]
        for b in range(BPC):
            out[cid * BPC + b] = o[32 * b:32 * b + 32, :].T
    return out


def kernel(**inputs) -> np.ndarray:
    in_maps, KC = prepare_inputs(**inputs)
    key = ("nc", KC)
    if key not in _cache:
        _cache[key] = build_module(KC)
    nc = _cache[key]
    res = bass_utils.run_bass_kernel_spmd(nc, in_maps,
                                          core_ids=list(range(NCORES)))
    return unshard_output(res.results)


# revision 4
# speedup vs baseline: 1.1198x; 1.1125x over previous
"""TRN2 Bass kernel v2 for nn_Attention_21758304322201 (sparse_attention).

Reference computation (B=32, L=2048, D=32, C=20):
    v = vals @ W_v.T
    k = LN(keys @ W_k.T);  q = LN(ques @ W_q.T)
    a = q @ k.T / sqrt(C);  a[masked keys] = -inf
    p = softmax(a);  o = p @ v
    out = LN(o + ques)

v2 restructure (vs the v1 baseline) — targets the dependency stalls that
dominated v1's trace (engines <35% busy on real work):
  * Batches are processed in PAIRS with block-diagonal stationaries: the
    score matmuls contract 64 rows (two 22-dim batch blocks) and cover half
    a key chunk of two batches at once; the o matmuls contract a mixed
    128-key partition layout with a block-diagonal vals/ones stationary.
    Safety: concurrent row-disjoint matmuls writing the SAME PSUM bank fault
    the HW (verified empirically) — here pair A (PE rows 0:64) targets bank
    0 and pair B (rows 64:128) bank 1; same-row matmuls serialize in HW.
  * One exp instruction per j over a [128, 1024] 2-bank scores tile
    (all 4 batches), rotated between ACT (native exp) and DVE (int16
    Schraudolph bf16-bit trick). Pool/GpSimd cannot read PSUM.
  * All row-stat broadcasts (LN rstd of q/k, output-LN mu/var) are produced
    pre-broadcast by 128-column indicator stationaries in the stats matmuls
    (PE stream cost depends only on N) -> no DRAM bounce, no broadcast DMAs.
  * The softmax normalizer rows are replicated 32x for free by ones-column
    blocks in the vals stationary; the s*q residual term reads them via a
    pre-arranged bf16 ques layout (quesT2). All elementwise work runs on
    ACT/DVE: the Pool/GpSimd engine measured ~3x slower end to end (poor Q7
    software pipelining + SBUF-port contention with the DVE).
  * Output LN is per-q-tile with centered moments (d1 = z - mu_bc,
    var = E[d1^2]); Ln/Exp are batched across groups of 4 q tiles to avoid
    ACT table-set thrash; g_o folds into the Exp bias (ln g).
  * ques/keys/vals ship as bf16 bits from the host (no on-device f32->bf16
    conversions; projection/stat matmuls stream bf16 anyway).
"""
import math

import numpy as np

from concourse import bacc, bass, bass_utils, tile
from concourse import mybir

dt = mybir.dt
F32 = dt.float32
BF16 = dt.bfloat16
I16 = dt.int16
U16 = dt.uint16
AO = mybir.AluOpType
AF = mybir.ActivationFunctionType

# problem constants (hardcoded per harness contract)
B, LQ, LK, D, C = 32, 2048, 2048, 32, 20
EPS = 1e-5
NCORES = 8
BPC = B // NCORES          # batches per core = 4
CAUG = C + 1               # 21-dim augmented projection (mean fold)
CDIM = C + 2               # +1 guard dim for pad masking
NT = 256                   # q-tile width
NQT = LQ // NT             # 8 q tiles
S20 = math.sqrt(C)
GS = 1.0 / S20             # score scale

# int16 Schraudolph (bf16 bit pattern): bits = round(x * A16 + B16)
A16 = 128.0 / math.log(2.0)
B16 = 127.0 * 128.0 - 5.6          # max rel err ~3.3%, mean ~1.8%

# exp engine rotation per j (one [128,1024] exp per j): 'A' = ACT, 'D' = DVE.
# STRICT alternation across j (including qt boundaries): adjacent j's must
# exp on different engines or the 2-deep scores pipeline serializes.
EXP_PATTERN = ("ADADADADA", "DADADADAD")

PHASES = 3
_cache: dict = {}


def _exp_engine(qt, j):
    pat = EXP_PATTERN[qt % len(EXP_PATTERN)]
    return pat[j % len(pat)]




def build_module(KC: int, reps: int = 1):
    """Build the SPMD bass module for per-core work. KC = padded key count."""
    NJ = KC // 128
    nc = bacc.Bacc("TRN2", target_bir_lowering=False, debug=False,
                   num_devices=NCORES)

    def din(name, shape, dty=F32):
        return nc.dram_tensor(name, shape, dty, kind="ExternalInput").ap()

    quesT_d = din("quesT", [128, LQ], U16)
    quesT2_d = din("quesT2", [128, 2 * LQ], U16)
    keysT_d = din("keysT", [128, KC], U16)
    valsP2_d = din("valsP2", [128, NJ * 512], U16)
    wq_d = din("wq_st", [128, 32])
    wk_d = din("wk_st", [128, 32])
    wvz_d = din("wvz", [128, 256])
    indmu_q_d = din("indmu_q", [128, 128])
    indmu_k_d = din("indmu_k", [128, 128])
    indsq_d = din("indsq", [128, 128])
    indz_mu_d = din("indz_mu", [128, 128])
    indz_sq_d = din("indz_sq", [128, 128])
    glb_d = din("glb", [128, 4])     # cols: g_o, b_o, ln(g_o), (pad)
    qrow1_d = nc.dram_tensor("qrow1", [4, LQ], U16, kind="ExternalInput").ap()
    krow1_d = nc.dram_tensor("krow1", [4, KC], U16, kind="ExternalInput").ap()
    out_d = nc.dram_tensor("out", [128, LQ], F32, kind="ExternalOutput").ap()

    with tile.TileContext(nc) as tc:
        with tc.tile_pool(name="inp", bufs=1) as inp, \
             tc.tile_pool(name="cst", bufs=1) as cst:
            # ---- load inputs (already bf16 where applicable) ----
            quesT_bf = inp.tile([128, LQ], BF16)
            nc.sync.dma_start(quesT_bf[:].bitcast(U16), quesT_d)
            quesT2 = inp.tile([128, 2 * LQ], BF16)
            nc.sync.dma_start(quesT2[:].bitcast(U16), quesT2_d)
            keysT_bf = inp.tile([128, KC], BF16)
            nc.sync.dma_start(keysT_bf[:].bitcast(U16), keysT_d)
            valsP2_bf = inp.tile([128, NJ * 512], BF16)
            nc.sync.dma_start(valsP2_bf[:].bitcast(U16), valsP2_d)
            wq_f = cst.tile([128, 32], F32)
            nc.sync.dma_start(wq_f[:], wq_d)
            wk_f = cst.tile([128, 32], F32)
            nc.sync.dma_start(wk_f[:], wk_d)
            wvz_f = cst.tile([128, 256], F32)
            nc.sync.dma_start(wvz_f[:], wvz_d)
            indmu_q_f = cst.tile([128, 128], F32)
            nc.sync.dma_start(indmu_q_f[:], indmu_q_d)
            indmu_k_f = cst.tile([128, 128], F32)
            nc.sync.dma_start(indmu_k_f[:], indmu_k_d)
            indsq_f = cst.tile([128, 128], F32)
            nc.sync.dma_start(indsq_f[:], indsq_d)
            indz_mu = cst.tile([128, 128], F32)
            nc.sync.dma_start(indz_mu[:], indz_mu_d)
            indz_sq_f = cst.tile([128, 128], F32)
            nc.sync.dma_start(indz_sq_f[:], indz_sq_d)
            glb = cst.tile([128, 4], F32)
            nc.sync.dma_start(glb[:], glb_d)

            # ---- one-time constant prep ----
            wq_bf = cst.tile([128, 32], BF16)
            nc.vector.tensor_copy(wq_bf[:], wq_f[:])
            wk_bf = cst.tile([128, 32], BF16)
            nc.vector.tensor_copy(wk_bf[:], wk_f[:])
            wvz_bf = cst.tile([128, 256], BF16)
            nc.vector.tensor_copy(wvz_bf[:], wvz_f[:])
            indmu_q_bf = cst.tile([128, 128], BF16)
            nc.vector.tensor_copy(indmu_q_bf[:], indmu_q_f[:])
            indmu_k_bf = cst.tile([128, 128], BF16)
            nc.vector.tensor_copy(indmu_k_bf[:], indmu_k_f[:])
            indsq_bf = cst.tile([128, 128], BF16)
            nc.vector.tensor_copy(indsq_bf[:], indsq_f[:])
            indz_sq_bf = cst.tile([128, 128], BF16)
            nc.vector.tensor_copy(indz_sq_bf[:], indz_sq_f[:])
            eps_t = cst.tile([128, 1], F32)
            nc.gpsimd.memset(eps_t[:], EPS)
            # paired-layout k stationary; zero blocks + guard rows persist
            # across reps (per-rep folds only write rows rb+0:21)
            ksc2 = cst.tile([128, 2 * KC], BF16)
            nc.gpsimd.memset(ksc2[:], 0.0)
            for b in range(4):
                rb = 32 * (b % 2) + 64 * (b // 2)
                m = b % 2
                gdst = ksc2[rb + 21:rb + 22, :].rearrange(
                    "p (c k) -> p c k", k=128)[:, :, 64 * m:64 * m + 64]
                nc.sync.dma_start(
                    gdst.bitcast(U16),
                    krow1_d[b:b + 1, :].rearrange("p (c k) -> p c k", k=64))

            pk = dict(
                NJ=NJ, quesT_bf=quesT_bf, quesT2=quesT2, keysT_bf=keysT_bf,
                valsP2_bf=valsP2_bf, ksc2=ksc2,
                wq_bf=wq_bf, wk_bf=wk_bf, wvz_bf=wvz_bf,
                indmu_q_bf=indmu_q_bf, indmu_k_bf=indmu_k_bf,
                indsq_bf=indsq_bf, indz_mu=indz_mu, indz_sq_bf=indz_sq_bf,
                glb=glb, eps_t=eps_t, out_d=out_d,
                qrow1_d=qrow1_d, krow1_d=krow1_d,
            )

            if reps == 1:
                _body(nc, tc, pk)
            elif reps == 0:
                pass
            else:
                with tc.For_i(0, reps, 1):
                    _body(nc, tc, pk)

    nc.compile()
    return nc


class _Ph1Side:
    """One side (q or k) of phase 1, chunk-steppable for k/q interleaving."""

    def __init__(self, nc, pk, wk, sb1, pools, src_bf, w_bf, indmu_bf, L, tg):
        self.nc, self.pk, self.L, self.tg = nc, pk, L, tg
        self.src_bf, self.w_bf, self.indmu_bf = src_bf, w_bf, indmu_bf
        self.hat_bf = wk.tile([128, L], BF16, tag=f"hat{tg}")
        self.sq_bf = sb1.tile([128, L], BF16, tag=f"sq{tg}")
        self.var_sb = sb1.tile([128, L], F32, tag=f"var{tg}")
        self.sb1 = sb1
        self.wk = wk
        self.pools = pools       # shared (projp, mup, sqp, musqp)
        self.chunks = list(range(0, L, 512))

    def step(self, t0):
        nc = self.nc
        w = min(512, self.L - t0)
        projp, mup, sqp, musqp = self.pools
        proj_ps = projp.tile([128, 512], F32, tag="pp")
        for b in range(4):
            nc.tensor.matmul(
                proj_ps[32 * b:32 * b + 32, :w],
                self.w_bf[32 * b:32 * b + 32, :],
                self.src_bf[32 * b:32 * b + 32, t0:t0 + w],
                start=True, stop=True,
                tile_position=(32 * b, 32 * b),
            )
        # PSUM -> SBUF bf16 (ACT) + elementwise square (DVE)
        nc.scalar.copy(self.hat_bf[:, t0:t0 + w], proj_ps[:, :w])
        nc.vector.tensor_tensor(self.sq_bf[:, t0:t0 + w],
                                self.hat_bf[:, t0:t0 + w],
                                self.hat_bf[:, t0:t0 + w], AO.mult)
        mu_ps = mup.tile([128, 512], F32, tag="mu")
        nc.tensor.matmul(mu_ps[:, :w], self.indmu_bf[:],
                         self.hat_bf[:, t0:t0 + w], start=True, stop=True)
        sq_ps = sqp.tile([128, 512], F32, tag="sq")
        nc.tensor.matmul(sq_ps[:, :w], self.pk["indsq_bf"][:],
                         self.sq_bf[:, t0:t0 + w], start=True, stop=True)
        musq = musqp.tile([128, 512], F32, tag="musq")
        nc.scalar.activation(musq[:, :w], mu_ps[:, :w], AF.Square)
        nc.vector.tensor_tensor(self.var_sb[:, t0:t0 + w], sq_ps[:, :w],
                                musq[:, :w], AO.subtract)

    def ln(self):
        nc, tg = self.nc, self.tg
        self.lnv = self.sb1.tile([128, self.L], F32, tag=f"lnv{tg}")
        nc.scalar.activation(self.lnv[:], self.var_sb[:], AF.Ln,
                             bias=self.pk["eps_t"][:])

    def fold(self):
        nc, tg = self.nc, self.tg
        r_bc = self.sb1.tile([128, self.L], BF16, tag=f"rbc{tg}")
        nc.scalar.activation(r_bc[:], self.lnv[:], AF.Exp, scale=-0.5)
        sc_bf = self.wk.tile([128, self.L], BF16, tag=f"sc{tg}")
        nc.vector.tensor_tensor(sc_bf[:], self.hat_bf[:], r_bc[:], AO.mult)
        return sc_bf


def _body(nc, tc, pk):
    """One full forward pass for this core's 4 batches."""
    NJ = pk["NJ"]
    KC = NJ * 128
    quesT2, valsP2_bf, ksc2 = pk["quesT2"], pk["valsP2_bf"], pk["ksc2"]
    wvz_bf = pk["wvz_bf"]
    indz_mu, indz_sq_bf = pk["indz_mu"], pk["indz_sq_bf"]
    glb, eps_t, out_d = pk["glb"], pk["eps_t"], pk["out_d"]

    with tc.tile_pool(name="work", bufs=1) as wk:
        # ================= phase 1: projections + LN folds =================
        with tc.tile_pool(name="ph1sb", bufs=1) as sb1:
            with tc.tile_pool(name="pps", bufs=3, space="PSUM") as projp, \
                 tc.tile_pool(name="mups", bufs=2, space="PSUM") as mup, \
                 tc.tile_pool(name="sqps", bufs=2, space="PSUM") as sqp, \
                 tc.tile_pool(name="musqs", bufs=3) as musqp:
                pools = (projp, mup, sqp, musqp)
                kside = _Ph1Side(nc, pk, wk, sb1, pools, pk["keysT_bf"],
                                 pk["wk_bf"], pk["indmu_k_bf"], KC, "k")
                qside = _Ph1Side(nc, pk, wk, sb1, pools, pk["quesT_bf"],
                                 pk["wq_bf"], pk["indmu_q_bf"], LQ, "q")
                # interleave k/q chunks: PE/ACT/DVE pipeline across the sides
                steps = []
                for i in range(max(len(kside.chunks), len(qside.chunks))):
                    if i < len(kside.chunks):
                        steps.append((kside, kside.chunks[i]))
                    if i < len(qside.chunks):
                        steps.append((qside, qside.chunks[i]))
                for side, t0 in steps:
                    side.step(t0)
            # k side finalizes first so scores can start ASAP; the fold
            # writes the paired block layout directly (strided DVE writes,
            # rows 0:21 only -- guard rows are pre-loaded constants)
            kside.ln()
            k_rbc = sb1.tile([128, KC], BF16, tag="krbc")
            nc.scalar.activation(k_rbc[:], kside.lnv[:], AF.Exp, scale=-0.5)
            for b in range(4):
                rb = 32 * (b % 2) + 64 * (b // 2)
                m = b % 2
                dst = ksc2[rb:rb + 21, :].rearrange(
                    "p (c k) -> p c k", k=128)[:, :, 64 * m:64 * m + 64]
                nc.vector.tensor_tensor(
                    dst,
                    kside.hat_bf[32 * b:32 * b + 21, :].rearrange(
                        "p (c k) -> p c k", k=64),
                    k_rbc[32 * b:32 * b + 21, :].rearrange(
                        "p (c k) -> p c k", k=64),
                    AO.mult)
            qside.ln()
            qsc_bf = qside.fold()
            for b in range(4):
                nc.sync.dma_start(
                    qsc_bf[32 * b + C + 1:32 * b + C + 2, :].bitcast(U16),
                    pk["qrow1_d"][b:b + 1, :])

        if PHASES < 2:
            return

        # ============ phase 2+3: attention + per-tile output LN ============
        with tc.tile_pool(name="scps", bufs=2, space="PSUM") as scps, \
             tc.tile_pool(name="oacc", bufs=2, space="PSUM") as oaccp, \
             tc.tile_pool(name="z1ps", bufs=1, space="PSUM") as z1p, \
             tc.tile_pool(name="muvr", bufs=1, space="PSUM") as muvrp, \
             tc.tile_pool(name="psb", bufs=6) as psb, \
             tc.tile_pool(name="obfp", bufs=6) as obfp, \
             tc.tile_pool(name="ep", bufs=3) as ep:

            o_banks_all = []   # per qt: o_pair (pair A cols 0:NT, B NT:2NT)

            def phase2(qt):
                t0 = qt * NT
                o_pair = oaccp.tile([128, 2 * NT], F32, tag="o")
                o_banks_all.append(o_pair)
                for j in range(NJ):
                    scb = scps.tile([128, 4 * NT], F32, tag="scb")
                    # emission order A1,B1,A2,B2: pair A (rows 0:64) -> bank0
                    # (cols 0:512), pair B (rows 64:128) -> bank1; same-row
                    # matmuls serialize, cross-pair ones hit distinct banks.
                    for pair, h in ((0, 0), (1, 0), (0, 1), (1, 1)):
                        r0 = 64 * pair
                        c0 = 512 * pair + 256 * h
                        nc.tensor.matmul(
                            scb[:, c0:c0 + 256],
                            ksc2[r0:r0 + 64,
                                 128 * (2 * j + h):128 * (2 * j + h) + 128],
                            qsc_bf[r0:r0 + 64, t0:t0 + NT],
                            start=True, stop=True,
                            tile_position=(r0, 0),
                        )
                    # single wide exp over the whole j tile
                    eng = _exp_engine(qt, j)
                    if eng == "A":
                        p_t = psb.tile([128, 4 * NT], BF16, tag="p")
                        nc.scalar.activation(p_t[:], scb[:], AF.Exp,
                                             bias=0.0, scale=float(GS))
                        is_int = False
                    else:
                        p_t = psb.tile([128, 4 * NT], I16, tag="p")
                        nc.vector.tensor_scalar(
                            p_t[:], scb[:],
                            float(GS * A16), float(B16), AO.mult, AO.add)
                        is_int = True
                    # o accumulation: block-diagonal vals/ones stationaries,
                    # mixed-batch key partitions, all at tile_position (0,0)
                    for pair, h in ((0, 0), (1, 0), (0, 1), (1, 1)):
                        c0 = 512 * pair + 256 * h
                        blk = 512 * j + 256 * pair + 128 * h
                        p_sl = p_t[:, c0:c0 + 256]
                        if is_int:
                            p_sl = p_sl.bitcast(BF16)
                        nc.tensor.matmul(
                            o_pair[:, NT * pair:NT * pair + NT],
                            valsP2_bf[:, blk:blk + 128],
                            p_sl,
                            start=(j == 0 and h == 0),
                            stop=(j == NJ - 1 and h == 1),
                            tile_position=(0, 0))

            GRP = 4            # q tiles per Ln/Exp batch (limits table loads)
            d1s = {}           # qt -> d1 tile (alive until its group finalizes)
            var_grp = ep.tile([128, GRP * NT], F32, tag="vgrp")
            var_grp2 = ep.tile([128, GRP * NT], F32, tag="vgrp")
            var_grps = [var_grp, var_grp2]

            def phase3(qt):
                """Everything except the Ln/Exp/zo finalize (see group_fin)."""
                t0 = qt * NT
                o_pair = o_banks_all[qt]
                # o -> SBUF bf16 for the z1 matmul (PE can't read PSUM)
                o_bf0 = obfp.tile([128, NT], BF16, tag="obf")
                nc.scalar.copy(o_bf0[:], o_pair[:, 0:NT])
                o_bf1 = obfp.tile([128, NT], BF16, tag="obf")
                nc.scalar.copy(o_bf1[:], o_pair[:, NT:2 * NT])
                # t = ques * s_bc on Pool: s replicas sit in o_bf rows
                # 32:64 / 96:128; quesT2 holds ques pre-arranged at those
                # partitions (SBUF x SBUF ops need equal base partitions).
                t_sb = ep.tile([128, NT], F32, tag="t")
                for b in range(4):
                    o_bf = o_bf0 if b < 2 else o_bf1
                    r0 = 32 if b % 2 == 0 else 96
                    q2c = t0 if b < 2 else LQ + t0
                    nc.vector.tensor_tensor(
                        t_sb[32 * b:32 * b + 32, :],
                        quesT2[r0:r0 + 32, q2c:q2c + NT],
                        o_bf[r0:r0 + 32, :], AO.mult)
                # z1 = W_v @ o_hat: two accumulating block-diagonal matmuls
                z1_ps = z1p.tile([128, NT], F32, tag="z1")
                nc.tensor.matmul(z1_ps[:], wvz_bf[:, 0:128], o_bf0[:],
                                 start=True, stop=False, tile_position=(0, 0))
                nc.tensor.matmul(z1_ps[:], wvz_bf[:, 128:256], o_bf1[:],
                                 start=False, stop=True, tile_position=(0, 0))
                z_f = ep.tile([128, NT], F32, tag="z")
                nc.vector.tensor_tensor(z_f[:], z1_ps[:], t_sb[:], AO.add)
                # output LN, centered moments, all broadcast via stationaries
                mu_ps = muvrp.tile([128, NT], F32, tag="mv")
                nc.tensor.matmul(mu_ps[:], indz_mu[:], z_f[:],
                                 start=True, stop=True)
                d1 = ep.tile([128, NT], F32, tag=f"d1_{qt % (GRP + 1)}")
                nc.vector.tensor_tensor(d1[:], z_f[:], mu_ps[:], AO.subtract)
                d1s[qt] = d1
                d1sq = ep.tile([128, NT], BF16, tag="d1sq")
                nc.vector.tensor_tensor(d1sq[:], d1[:], d1[:], AO.mult)
                var_ps = muvrp.tile([128, NT], F32, tag="mv")
                nc.tensor.matmul(var_ps[:], indz_sq_bf[:], d1sq[:],
                                 start=True, stop=True)
                # stage var to SBUF (Copy needs no table load) for batched Ln
                g0 = (qt // GRP) * GRP
                nc.scalar.copy(
                    var_grps[(qt // GRP) % 2][:, (qt - g0) * NT:
                                              (qt - g0 + 1) * NT],
                    var_ps[:])

            def group_fin(g0):
                """Batched Ln/Exp + final normalize for q tiles g0..g0+GRP-1."""
                vg = var_grps[(g0 // GRP) % 2]
                lnv = ep.tile([128, GRP * NT], F32, tag="lnv")
                nc.scalar.activation(lnv[:], vg[:], AF.Ln, bias=eps_t[:])
                rstdg = ep.tile([128, GRP * NT], F32, tag="rstdg")
                nc.scalar.activation(rstdg[:], lnv[:], AF.Exp, scale=-0.5,
                                     bias=glb[:, 2:3])
                for qt in range(g0, g0 + GRP):
                    c0 = (qt - g0) * NT
                    zo = ep.tile([128, NT], F32, tag="zo")
                    nc.vector.tensor_tensor(zo[:], d1s[qt][:],
                                            rstdg[:, c0:c0 + NT], AO.mult)
                    nc.sync.dma_start(out_d[:, qt * NT:qt * NT + NT], zo[:])

            for qt in range(NQT):
                phase2(qt)
                if PHASES >= 3:
                    if qt > 0:
                        phase3(qt - 1)
                    if qt % GRP == 0 and qt >= GRP:
                        group_fin(qt - GRP)
            if PHASES >= 3:
                phase3(NQT - 1)
                group_fin(NQT - GRP)


# ---------------------------------------------------------------------------
# host side
# ---------------------------------------------------------------------------

def _bf16_bits(x):
    u = np.ascontiguousarray(x, np.float32).view(np.uint32)
    return ((u + 0x7FFF + ((u >> 16) & 1)) >> 16).astype(np.uint16)


def prepare_inputs(vals, keys, ques, key_mask, W_v, W_k, W_q,
                   g_k, b_k, g_q, b_q, g_o, b_o):
    """Shard + lay out the full inputs for the 8 cores. Returns (in_maps, KC)."""
    vals = np.ascontiguousarray(vals, np.float32)
    keys = np.ascontiguousarray(keys, np.float32)
    ques = np.ascontiguousarray(ques, np.float32)
    key_mask = np.asarray(key_mask)
    W_v = np.asarray(W_v, np.float32)
    W_k = np.asarray(W_k, np.float32)
    W_q = np.asarray(W_q, np.float32)
    g_k = np.asarray(g_k, np.float32)
    b_k = np.asarray(b_k, np.float32)
    g_q = np.asarray(g_q, np.float32)
    b_q = np.asarray(b_q, np.float32)
    g_o = np.asarray(g_o, np.float32)
    b_o = np.asarray(b_o, np.float32)

    # supported parameterization (holds for the harness inputs)
    if not (np.allclose(b_k, 0) and np.allclose(b_q, 0)):
        raise NotImplementedError("nonzero k/q LN bias not supported")
    if not (np.allclose(g_k, g_k.flat[0]) and np.allclose(g_q, g_q.flat[0])):
        raise NotImplementedError("non-uniform k/q LN gain not supported")
    if not (np.allclose(b_o, 0) and np.all(g_o > 0)):
        raise NotImplementedError("output LN with b_o!=0 or g_o<=0")
    guni = float(g_k.flat[0] * g_q.flat[0])

    counts = (~key_mask).sum(axis=1)
    KC = int(np.ceil(max(int(counts.max()), 1) / 128) * 128)
    NJ = KC // 128

    wq_aug = np.zeros((32, 32), np.float32)
    wq_aug[:, :C] = W_q.T
    wq_aug[:, C] = W_q.sum(axis=0) / S20
    wq_aug *= guni     # fold uniform LN gains into the q side
    wk_aug = np.zeros((32, 32), np.float32)
    wk_aug[:, :C] = W_k.T
    wk_aug[:, C] = -W_k.sum(axis=0) / S20

    wq_st = np.zeros((128, 32), np.float32)
    wk_st = np.zeros((128, 32), np.float32)
    wvz = np.zeros((128, 256), np.float32)
    # block-diagonal W_v for the merged z1 matmuls: o_bf0 rows (b0 dims @0:32,
    # b1 dims @64:96) -> z1 rows 0:32 / 32:64; o_bf1 -> z1 rows 64:96 / 96:128
    wvz[0:32, 0:32] = W_v.T
    wvz[64:96, 32:64] = W_v.T
    wvz[0:32, 128 + 64:128 + 96] = W_v.T
    wvz[64:96, 128 + 96:128 + 128] = W_v.T
    indmu_q = np.zeros((128, 128), np.float32)
    indmu_k = np.zeros((128, 128), np.float32)
    indsq = np.zeros((128, 128), np.float32)
    indz_mu = np.zeros((128, 128), np.float32)
    indz_sq = np.zeros((128, 128), np.float32)
    glb = np.zeros((128, 4), np.float32)
    for b in range(BPC):
        r = 32 * b
        wq_st[r:r + 32] = wq_aug
        wk_st[r:r + 32] = wk_aug
        for c in range(CAUG + 1):
            indmu_q[r + C, r + c] = S20 / C
            indmu_k[r + C, r + c] = -S20 / C
            indsq[r:r + C, r + c] = 1.0 / C
        indz_mu[r:r + 32, r:r + 32] = 1.0 / D
        indz_sq[r:r + 32, r:r + 32] = 1.0 / D
        glb[r:r + 32, 0] = g_o
        glb[r:r + 32, 1] = b_o
        glb[r:r + 32, 2] = np.log(g_o)

    in_maps = []
    for cid in range(NCORES):
        quesT = np.zeros((128, LQ), np.uint16)
        quesT2 = np.zeros((128, 2 * LQ), np.uint16)
        keysT = np.zeros((128, KC), np.uint16)
        valsP2 = np.zeros((128, NJ * 512), np.uint16)
        krow1 = np.zeros((4, KC), np.uint16)
        one_bits = np.uint16(0x3F80)
        for b in range(BPC):
            g = cid * BPC + b
            idx = np.flatnonzero(~key_mask[g])
            ci = len(idx)
            qb = _bf16_bits(ques[g].T)
            quesT[32 * b:32 * b + 32] = qb
            r0 = 32 if b % 2 == 0 else 96
            c0 = 0 if b < 2 else LQ
            quesT2[r0:r0 + 32, c0:c0 + LQ] = qb
            keysT[32 * b:32 * b + 32, :ci] = _bf16_bits(keys[g][idx].T)
            vc = np.zeros((KC, D), np.float32)
            vc[:ci] = vals[g][idx]
            vcb = _bf16_bits(vc)
            pair, half = b // 2, b % 2
            for j in range(NJ):
                for h in range(2):
                    blk = 512 * j + 256 * pair + 128 * h
                    rows = slice(64 * half, 64 * half + 64)
                    cols = slice(blk + 64 * half, blk + 64 * half + 32)
                    valsP2[rows, cols] = vcb[128 * j + 64 * h:
                                             128 * j + 64 * h + 64]
                    valsP2[rows, cols.stop:cols.stop + 32] = one_bits
            krow1[b, ci:] = np.float32(-300.0).view(np.uint32) >> 16
        in_maps.append({
            "quesT": quesT, "quesT2": quesT2, "keysT": keysT,
            "valsP2": valsP2,
            "wq_st": wq_st, "wk_st": wk_st, "wvz": wvz,
            "indmu_q": indmu_q, "indmu_k": indmu_k, "indsq": indsq,
            "indz_mu": indz_mu, "indz_sq": indz_sq, "glb": glb,
            "qrow1": np.full((4, LQ), 0x3F80, np.uint16),
            "krow1": krow1,
        })
    return in_maps, KC


def unshard_output(results):
    out = np.empty((B, LQ, D), np.float32)
    for cid in range(NCORES):
        o = results[cid]["out"]
        for b in range(BPC):
            out[cid * BPC + b] = o[32 * b:32 * b + 32, :].T
    return out


def kernel(**inputs) -> np.ndarray:
    in_maps, KC = prepare_inputs(**inputs)
    key = ("nc", KC)
    if key not in _cache:
        _cache[key] = build_module(KC)
    nc = _cache[key]
    res = bass_utils.run_bass_kernel_spmd(nc, in_maps,
                                          core_ids=list(range(NCORES)))
    return unshard_output(res.results)
